# revision 21
# baseline (speedup 1.0000x reference)
"""BrainGNN message-passing kernel for Trainium2 (Bass/Tile), SPMD over 8 cores.

Strategy
--------
Phase 1 (node MLP, sharded by node range): each core computes
    h   = relu(pseudo @ W1)                       [n, 8]
    xt  = einsum('nr,nrd->nd', x, (h @ W2 + b2).reshape(n, R, D1))
reformulated as xt[n,d] = sum_k h'[n,k] * (x @ W2aug[:,k,:])[n,d]; when b2 == 0
(the spec fill) the ones-augmentation column is dropped (KA=8). Inputs are
single-plane bf16 (rel err ~3e-3 << 2e-2 tolerance); matmuls accumulate in
fp32 PSUM; x/pseudo are interleaved per 128-node tile in one DRAM tensor so a
few large DMAs feed the whole phase; xt is written back as bf16.

Host assembles the 8 slices into a [N, 128]-bf16 table (only cols 0:32 live;
rows 256 B apart - the gather stride granularity is 256 B, the payload is not).

Phase 2 (edges, sharded by dst range): host packs, per core, the incoming
edges (+ self loops) of each dst node into a dense padded layout:
dst nodes sorted by degree desc, grouped 128 at a time, each group padded to
its max degree Mg (shared across cores so the SPMD program is identical).
On device: per-group segment softmax of the edge weights is precomputed
upfront and folded with 1/sum into e_norm (bf16), which the Scalar engine
expands to e_rep[p, j*32+d] = e_norm[p, j]; gathers fetch only the 64 B bf16
payload of each neighbor row (elem_size=32 bf16, elem_step=128 -> stride
256 B) into per-group tiles; one fully-contiguous packed-bf16 DVE multiply
per group (so the GPSIMD descriptor generators are barely ever locked out of
the shared SBUF port), then a 1-port strided reduce per group. Bias is added
on the host.
"""

import os

import numpy as np

import concourse.bass as bass
import concourse.bacc as bacc
import concourse.tile as tile
from concourse import mybir
from concourse.bass_utils import run_bass_kernel_spmd

F32 = mybir.dt.float32
BF16 = mybir.dt.bfloat16
I16 = mybir.dt.int16
AF = mybir.ActivationFunctionType
ALU = mybir.AluOpType
AX = mybir.AxisListType

N, R, K, D1 = 25600, 200, 8, 32
E = 819200
NCORES = 8
NL = N // NCORES            # 3200 dst nodes per core
P = 128
NGROUPS = NL // P           # 25
TBLW = 128                  # xt table row stride in bf16 (256 B, gather stride unit)
GELEM = 32                  # gathered payload per edge in bf16 (64 B)
GSUB = 20                   # max j-columns per gather window
WCOLS = 55                  # idx plane padding unit (trailing -1 cols)
NEG = -1.0e30


def _dma_gather_raw(gp_eng, out_ap, in_ap, idxs_ap, num_idxs, num_idxs_reg,
                    elem_size, elem_step, queue_num):
    """nc.gpsimd.dma_gather minus the over-conservative elem_size%256 assert
    (the ISA encodes elem_size as a free uint16 element count; only the row
    stride is 256-B-granular, and the Q7 ucode's 256B elem check exists only
    on the transpose path). Takes a caller-owned num_idxs register so that
    all gathers share ONE register: per-gather to_reg() writes get guarded
    against prior readers' DMA completion when physical registers recycle,
    which serializes descriptor generation into ~10us-barrier rounds."""
    from concourse import ap_utils

    gp_eng._assert_queue_num(queue_num)
    assert idxs_ap.dtype == mybir.dt.int16
    assert in_ap.dtype == out_ap.dtype
    assert ap_utils.ap_is_contiguous(in_ap.ap[1:])
    assert ap_utils.ap_is_contiguous(out_ap.ap[1:])
    assert ap_utils.ap_is_contiguous(idxs_ap.ap[1:])
    assert in_ap.ap[-1][1] == out_ap.ap[-1][1] == elem_size
    assert out_ap.ap[0][1] * out_ap.ap[1][1] == ((num_idxs + 127) // 128) * 128
    assert in_ap.ap[0][0] == elem_step
    stride_bytes = elem_step * mybir.dt.size(in_ap.dtype)
    stride_bytes_256 = stride_bytes // 256
    assert stride_bytes_256 * 256 == stride_bytes and stride_bytes_256 < 256

    _in_ap = gp_eng.lower_ap_dma(in_ap, for_custom_bir_dma=True)
    _idxs_ap = gp_eng.lower_ap(idxs_ap)
    _out_ap = gp_eng.lower_ap(out_ap)
    return gp_eng.add_instruction(
        mybir.InstDMAGatherAnt(
            name=gp_eng.bass.get_next_instruction_name(),
            ins=[
                *_in_ap,
                _idxs_ap,
                gp_eng.lower_val_access(num_idxs_reg),
            ],
            outs=[_out_ap],
            transpose=False,
            num_idxs=num_idxs,
            elem_size=elem_size,
            stride_bytes_256=stride_bytes_256,
            gen_mode=0,
            single_packet=False,
            queue_num=queue_num,
            sbuf_tokens_per_rank=0,
            sbuf_free_dim_per_rank=0,
            sbuf_free_dim_pad_per_rank=0,
            sbuf_byte_offset=0,
        )
    )


# ---------------------------------------------------------------- phase 1

def _build_phase1(ka):
    cw = ka * D1
    nc = bacc.Bacc("TRN2", target_bir_lowering=False, debug=False)
    dat_d = nc.dram_tensor("dat", [R, 2 * NL], BF16, kind="ExternalInput").ap()
    wts_d = nc.dram_tensor("wts", [R, K + cw], BF16, kind="ExternalInput").ap()
    xtout = nc.dram_tensor("xtout", [NL, D1], BF16, kind="ExternalOutput").ap()

    OB = 5  # tiles batched per output DMA

    with tile.TileContext(nc) as tc:
        with (
            tc.tile_pool(name="big", bufs=1) as big,
            tc.tile_pool(name="wp", bufs=1) as wp,
            tc.tile_pool(name="hp", bufs=3) as hp,
            tc.tile_pool(name="tp", bufs=3) as tp,
            tc.tile_pool(name="op", bufs=2) as op,
            tc.tile_pool(name="pph", bufs=3, space="PSUM") as pph,
            tc.tile_pool(name="ppg", bufs=5, space="PSUM") as ppg,
        ):
            wa = wp.tile([128, K + cw], BF16, tag="wa")
            wb = wp.tile([72, K + cw], BF16, tag="wb")
            da = big.tile([128, 2 * NL], BF16, tag="da")
            db = big.tile([72, 2 * NL], BF16, tag="db")

            nc.sync.dma_start(out=wa[:], in_=wts_d[0:128, :])
            nc.sync.dma_start(out=wb[:], in_=wts_d[128:200, :])
            # data in three column chunks (aligned to 256-col tile pairs)
            cuts = [0, 2 * 256, 13 * 256, 2 * NL]
            for lo, hi in zip(cuts[:-1], cuts[1:]):
                cs = slice(lo, hi)
                nc.sync.dma_start(out=da[:, cs], in_=dat_d[0:128, cs])
                nc.sync.dma_start(out=db[:, cs], in_=dat_d[128:200, cs])

            obuf = None
            for t in range(NGROUPS):
                ps_ = slice(256 * t, 256 * t + 128)
                xs_ = slice(256 * t + 128, 256 * t + 256)
                ph = pph.tile([P, K], F32, tag="ph")
                nc.tensor.matmul(out=ph[:], lhsT=da[:, ps_], rhs=wa[:, 0:K],
                                 start=True, stop=False)
                nc.tensor.matmul(out=ph[:], lhsT=db[:, ps_], rhs=wb[:, 0:K],
                                 start=False, stop=True)
                pg = ppg.tile([P, cw], F32, tag="pg")
                nc.tensor.matmul(out=pg[:], lhsT=da[:, xs_], rhs=wa[:, K:],
                                 start=True, stop=False)
                nc.tensor.matmul(out=pg[:], lhsT=db[:, xs_], rhs=wb[:, K:],
                                 start=False, stop=True)

                h = hp.tile([P, ka], F32, tag="h")
                if ka > K and t < 3:
                    nc.vector.memset(h[:, K:ka], 1.0)
                nc.scalar.activation(out=h[:, 0:K], in_=ph[:], func=AF.Relu)

                # tmp[p, d, k] = pg[p, k*D1+d] * h[p, k]; then reduce over k
                tmp = tp.tile([P, cw], BF16, tag="tmp")
                in0 = pg[:].rearrange("p (k d) -> p d k", k=ka)
                hap = h[:]
                in1 = bass.AP(tensor=hap.tensor, offset=hap.offset,
                              ap=[hap.ap[0], [0, D1], hap.ap[1]])
                tview = tmp[:].rearrange("p (d k) -> p d k", d=D1)
                nc.vector.tensor_tensor(out=tview, in0=in0, in1=in1, op=ALU.mult)
                if t % OB == 0:
                    obuf = op.tile([P, OB * D1], F32, tag="ob")
                nc.vector.reduce_sum(out=obuf[:, (t % OB) * D1:(t % OB + 1) * D1],
                                     in_=tview, axis=AX.X)
                if t % OB == OB - 1:
                    t0 = t - (OB - 1)
                    obb = op.tile([P, OB * D1], BF16, tag="obb")
                    nc.scalar.activation(out=obb[:], in_=obuf[:], func=AF.Copy)
                    dst = xtout[t0 * P:(t0 + OB) * P, :]
                    nc.sync.dma_start(
                        out=dst.rearrange("(t p) d -> p t d", p=P),
                        in_=obb[:].rearrange("p (t d) -> p t d", t=OB))
    nc.compile()
    return nc


# ---------------------------------------------------------------- phase 2

def _build_phase2(mgs, sewp):
    SEW = int(sum(mgs))
    nc = bacc.Bacc("TRN2", target_bir_lowering=False, debug=False,
                   num_swdge_queues=4)
    xt = nc.dram_tensor("xt", [N, TBLW], BF16, kind="ExternalInput").ap()
    ew = nc.dram_tensor("ew", [P, SEW], F32, kind="ExternalInput").ap()
    idx = nc.dram_tensor("idx", [P, 8 * sewp], I16, kind="ExternalInput").ap()
    out = nc.dram_tensor("out", [NL, D1], F32, kind="ExternalOutput").ap()

    off_g = np.concatenate([[0], np.cumsum(mgs)]).astype(int)

    # gather windows: split each group into <=GSUB-wide column windows,
    # rotated across the 4 SWDGE queues (best-measured pipeline structure:
    # ~2.2 ns/descriptor aggregate across the 4 Q7 descriptor-generator
    # pairs, which are the bottleneck of this phase)
    wins = []  # (g, w0, wlen, last)
    for g in range(NGROUPS):
        mg = int(mgs[g])
        nwin = -(-mg // GSUB)
        step = -(-mg // nwin)
        for w0 in range(0, mg, step):
            wl = min(step, mg - w0)
            wins.append((g, w0, wl, w0 + wl == mg))

    # idx arrives in chunks of several groups so early gathers start fast
    idx_cut_groups = [0, 2, 5, 9, 13, 17, 21, NGROUPS]

    with tile.TileContext(nc) as tc:
        with (
            tc.tile_pool(name="const", bufs=1) as const,
            tc.tile_pool(name="gp", bufs=6) as gp,
            tc.tile_pool(name="sp", bufs=8) as sp,
            tc.tile_pool(name="ep", bufs=3) as ep,
            tc.tile_pool(name="tp", bufs=3) as tp,
            tc.tile_pool(name="op", bufs=3) as op,
        ):
            ew_all = const.tile([P, SEW], F32, tag="ew_all")
            nc.sync.dma_start(out=ew_all[:], in_=ew[:, :])
            idx_all = const.tile([P, 8 * sewp], I16, tag="idx_all")
            for glo, ghi in zip(idx_cut_groups[:-1], idx_cut_groups[1:]):
                a, b = 8 * int(off_g[glo]), 8 * int(off_g[ghi])
                nc.sync.dma_start(out=idx_all[:, a:b], in_=idx[:, a:b])

            # upfront segment softmax for all groups, folded with 1/sum:
            # e_norm = exp(ew - max) / sum(exp(ew - max)); pads -> 0
            e_all = const.tile([P, SEW], F32, tag="e_all")
            en_all = const.tile([P, SEW], BF16, tag="en_all")
            for g in range(NGROUPS):
                o, mg = int(off_g[g]), int(mgs[g])
                ewt = ew_all[:, o:o + mg]
                mneg = sp.tile([P, 1], F32, tag="mneg")
                nc.vector.reduce_max(out=mneg[:], in_=ewt, axis=AX.X,
                                     negate=True)
                et = e_all[:, o:o + mg]
                nc.scalar.activation(out=et, in_=ewt, func=AF.Exp,
                                     bias=mneg[:, 0:1], scale=1.0)
                s = sp.tile([P, 1], F32, tag="s")
                nc.vector.reduce_sum(out=s[:], in_=et, axis=AX.X)
                sr = sp.tile([P, 1], F32, tag="sr")
                nc.vector.reciprocal(out=sr[:], in_=s[:])
                nc.scalar.activation(out=en_all[:, o:o + mg], in_=et,
                                     func=AF.Copy, scale=sr[:, 0:1])

            # one shared num_idxs register per distinct window width, written
            # once up front: per-gather to_reg() would allocate ~53 virtual
            # registers whose physical recycling guards each new write against
            # prior gathers' DMA completion, quantizing the stream into
            # ~10us-barrier rounds
            wregs = {wl: nc.gpsimd.to_reg(P * wl)
                     for wl in sorted({w[2] for w in wins})}

            # gather + consume pipeline
            gt = None
            for wi, (g, w0, wl, last) in enumerate(wins):
                o, mg = int(off_g[g]), int(mgs[g])
                a0 = o + w0
                if w0 == 0:
                    gt = gp.tile([P, mg * GELEM], BF16, tag="gather")
                _dma_gather_raw(
                    nc.gpsimd,
                    out_ap=gt[:, w0 * GELEM:(w0 + wl) * GELEM]
                        .rearrange("p (j d) -> p j d", d=GELEM),
                    in_ap=xt[:, 0:GELEM],
                    idxs_ap=idx_all[:, 8 * a0:8 * (a0 + wl)],
                    num_idxs=P * wl,
                    num_idxs_reg=wregs[wl],
                    elem_size=GELEM, elem_step=TBLW, queue_num=wi % 4)

                if last:
                    # e_rep[p, j*32+d] = e_norm[p, j] on the Scalar engine
                    # (own SBUF ports - no GPSIMD lockout)
                    erep = ep.tile([P, mg * D1], BF16, tag="erep")
                    enap = en_all[:, o:o + mg]
                    in_r = bass.AP(tensor=enap.tensor, offset=enap.offset,
                                   ap=[enap.ap[0], enap.ap[1], [0, D1]])
                    nc.scalar.activation(
                        out=erep[:].rearrange("p (j d) -> p j d", d=D1),
                        in_=in_r, func=AF.Copy)
                    # one packed-bf16 multiply per group (short DVE lockout)
                    tmp = tp.tile([P, mg * D1], BF16, tag="tmp")
                    nc.vector.tensor_tensor(out=tmp[:], in0=gt[:],
                                            in1=erep[:], op=ALU.mult)
                    # 1-port strided reduce over j (no GPSIMD lockout)
                    ot = op.tile([P, D1], F32, tag="o")
                    nc.vector.reduce_sum(
                        out=ot[:],
                        in_=tmp[:].rearrange("p (j d) -> p d j", d=D1),
                        axis=AX.X)
                    nc.sync.dma_start(out=out[g * P:(g + 1) * P, :],
                                      in_=ot[:])
    nc.compile()
    return nc


# ---------------------------------------------------------------- host prep

def _prep_phase1_inputs(x, pseudo, W1, W2, b2, ka):
    import ml_dtypes
    bf16 = ml_dtypes.bfloat16

    W2rkd = np.ascontiguousarray(
        W2.reshape(K, R, D1).transpose(1, 0, 2)).reshape(R, K * D1)
    if ka > K:
        W2aug = np.concatenate([W2rkd, b2.reshape(R, D1)], axis=1)
    else:
        W2aug = W2rkd
    wts = np.ascontiguousarray(
        np.concatenate([W1, W2aug], axis=1).astype(np.float32).astype(bf16))
    in_maps = []
    for c in range(NCORES):
        sl = slice(c * NL, (c + 1) * NL)
        ps3 = pseudo[sl].T.reshape(R, NGROUPS, P)
        xs3 = x[sl].T.reshape(R, NGROUPS, P)
        dat = np.ascontiguousarray(
            np.concatenate([ps3, xs3], axis=2).reshape(R, 2 * NL).astype(bf16))
        in_maps.append(dict(dat=dat, wts=wts))
    return in_maps


def _prep_edges(edge_index, edge_weight):
    """Pack edges (+ self loops) into the padded per-core layout.

    dst nodes are sorted by (in-)degree globally and dealt round-robin to the
    8 cores, so every core's group g has near-identical degree profile: the
    shared pad width Mg[g] (= degree at global rank g*1024) is tight and the
    per-core slot counts are balanced.

    Returns (mgs, EWs, IDXs, node_of_row): group pad widths (shared), per-core
    edge-weight planes [128, SEW], wrapped int16 index planes [128, 8*SEW],
    and per-core arrays mapping output row -> global node id.
    """
    src = edge_index[0].astype(np.int64)
    dst = edge_index[1].astype(np.int64)
    loops = np.arange(N, dtype=np.int64)
    src_all = np.concatenate([src, loops])
    dst_all = np.concatenate([dst, loops])
    w_all = np.concatenate([edge_weight.astype(np.float32),
                            np.ones(N, np.float32)])

    deg_all = np.bincount(dst_all, minlength=N)
    order_global = np.argsort(-deg_all, kind="stable")
    rank_of = np.empty(N, np.int64)
    rank_of[order_global] = np.arange(N)
    deg_by_rank = deg_all[order_global]

    mgs = [int(deg_by_rank[g * P * NCORES]) for g in range(NGROUPS)]
    SEW = int(sum(mgs))
    SEWP = -(-SEW // WCOLS) * WCOLS
    off_ew = np.concatenate([[0], np.cumsum(mgs)])[:-1].astype(np.int64)

    rk = rank_of[dst_all]
    core = rk % NCORES
    q_all = rk // NCORES          # per-core row position 0..NL-1

    EWs, IDXs, node_of_row = [], [], []
    for c in range(NCORES):
        m = core == c
        s_c, q_c, w_c = src_all[m], q_all[m], w_all[m]
        o = np.argsort(q_c, kind="stable")
        q_s, s_s, w_s = q_c[o], s_c[o], w_c[o]
        deg_c = deg_by_rank[np.arange(NL) * NCORES + c]
        starts = np.concatenate([[0], np.cumsum(deg_c)])
        j = np.arange(len(o)) - starts[q_s]
        g_arr = q_s // P
        p_arr = q_s % P

        EW = np.full((P, SEW), NEG, np.float32)
        EW[p_arr, off_ew[g_arr] + j] = w_s

        slot = j * P + p_arr
        IDX16 = np.zeros((16, 8 * SEWP), np.int16)
        IDX16[:, 8 * SEW:] = -1
        IDX16[slot % 16, off_ew[g_arr] * 8 + slot // 16] = s_s.astype(np.int16)
        EWs.append(EW)
        IDXs.append(np.tile(IDX16, (8, 1)))
        node_of_row.append(order_global[np.arange(NL) * NCORES + c])
    return mgs, SEWP, EWs, IDXs, node_of_row


# ---------------------------------------------------------------- entry

LAST_STATS = {}


def _run(nc, in_maps, core_ids, label):
    trace = bool(os.environ.get("BGNN_TRACE"))
    res = run_bass_kernel_spmd(nc, in_maps, core_ids=core_ids, trace=trace)
    LAST_STATS[label] = res.exec_time_ns
    return res


def kernel(x, pseudo, edge_index, edge_weight, W1, W2, b2, bias):
    core_ids = list(range(NCORES))

    # phase 1: xt table
    ka = K if not np.any(b2) else K + 1
    nc1 = _build_phase1(ka)
    in_maps1 = _prep_phase1_inputs(x, pseudo, W1, W2, b2, ka)
    res1 = _run(nc1, in_maps1, core_ids, "phase1")
    import ml_dtypes
    XT = np.zeros((N, TBLW), ml_dtypes.bfloat16)
    XT[:, 0:D1] = np.concatenate(
        [res1.results[c]["xtout"] for c in range(NCORES)], axis=0)

    # phase 2: edges
    mgs, sewp, EWs, IDXs, node_of_row = _prep_edges(edge_index, edge_weight)
    nc2 = _build_phase2(mgs, sewp)
    in_maps2 = [dict(xt=XT, ew=EWs[c], idx=IDXs[c]) for c in range(NCORES)]
    res2 = _run(nc2, in_maps2, core_ids, "phase2")

    out_full = np.empty((N, D1), np.float32)
    for c in range(NCORES):
        out_full[node_of_row[c]] = res2.results[c]["out"]
    out_full += bias.astype(np.float32)
    return out_full
